# revision 14
# baseline (speedup 1.0000x reference)
"""Trainium2 Bass kernel for AttnProcessor self-attention (B=2,S=2048,C=1024,H=16).

Sharding: 8 cores, core c owns heads (2c, 2c+1) for both batches (tensor
parallel on the head dim for QKV); attention outputs are redistributed with
two 8-core AllToAlls (one per batch, the first hidden under batch-1 compute)
so core c computes the output projection + residual for output rows
(b=c//4, s in [512*(c%4), 512*(c%4+1))). Host picks out1/out2 per core.

Per-core pipeline (all matmuls fp32r):
  qT/kT projections in [c'=128, s] layout, v' in [s, 2x(64+ones)] layout
  (padded to 256 free for fp32r full rate), row-tiled (64x128) QK^T per head
  pair, exp on ScalarE (scale=1/8, no max subtraction -- scores are O(5)),
  PV accumulates V'.T @ probsT giving both the attention output (transposed)
  and the softmax denominators (ones row). Denominator reciprocals are
  computed batched pre-collective and ride the AllToAll; normalization +
  output projection run per received chunk.
"""
import numpy as np

import concourse.bacc as bacc
import concourse.bass as bass
import concourse.tile as tile
import concourse.tile_rust as tile_rust
from concourse import mybir
from concourse.bass_utils import run_bass_kernel_spmd

F32 = mybir.dt.float32
F32R = mybir.dt.float32r

B, S, C, H, D = 2, 2048, 1024, 16, 64
N_CORES = 8
BS = B * S  # 4096
SCALE = 1.0 / np.sqrt(D)

_CACHE = {}


def _build():
    nc = bacc.Bacc(num_devices=N_CORES)
    hsT = nc.declare_dram_parameter("hsT", [C, BS], F32R, isOutput=False)
    wq = nc.declare_dram_parameter("wq", [C, 128], F32R, isOutput=False)
    wk = nc.declare_dram_parameter("wk", [C, 128], F32R, isOutput=False)
    wv = nc.declare_dram_parameter("wv", [C, 256], F32R, isOutput=False)
    wo = nc.declare_dram_parameter("wo", [C, C], F32R, isOutput=False)
    bqk = nc.declare_dram_parameter("bqk", [128, 2], F32, isOutput=False)
    bvb = nc.declare_dram_parameter("bvb", [1, 256], F32, isOutput=False)
    res = nc.declare_dram_parameter("res", [512, C], F32, isOutput=False)
    out1 = nc.declare_dram_parameter("out1", [512, C], F32, isOutput=True)
    out2 = nc.declare_dram_parameter("out2", [512, C], F32, isOutput=True)

    with tile.TileContext(nc) as tc:
        with (
            tc.tile_pool(name="wpool", bufs=1) as wpool,
            tc.tile_pool(name="hpool", bufs=1) as hpool,
            tc.tile_pool(name="qkpool", bufs=2) as qkpool,
            tc.tile_pool(name="ppool", bufs=3) as ppool,
            tc.tile_pool(name="spool", bufs=3) as spool,
            tc.tile_pool(name="opool", bufs=2) as opool,
            tc.tile_pool(name="psum", bufs=1, space="PSUM") as psum,
            tc.tile_pool(name="dram", bufs=1, space="DRAM") as dram,
        ):
            # ---- weight / constant / input loads (hsT early: long pole) ----
            wq_sb, wk_sb, wv_sb, wo_sb = [], [], [], []
            for cc in range(8):
                t = wpool.tile([128, 128], F32R, tag=f"wq{cc}")
                nc.scalar.dma_start(out=t[:], in_=wq[128 * cc:128 * (cc + 1), :])
                wq_sb.append(t)
            hs0 = []
            for cc in range(8):
                t = hpool.tile([128, 2048], F32R, tag=f"hs{cc}", name=f"hs0_{cc}")
                hs0.append(t)
            for g in range(4):
                for cc in range(8):
                    nc.scalar.dma_start(
                        out=hs0[cc][:, 512 * g:512 * (g + 1)],
                        in_=hsT[128 * cc:128 * (cc + 1),
                                512 * g:512 * (g + 1)])
            for cc in range(8):
                t = wpool.tile([128, 128], F32R, tag=f"wk{cc}")
                nc.scalar.dma_start(out=t[:], in_=wk[128 * cc:128 * (cc + 1), :])
                wk_sb.append(t)
                t = wpool.tile([128, 256], F32R, tag=f"wv{cc}")
                nc.scalar.dma_start(out=t[:], in_=wv[128 * cc:128 * (cc + 1), :])
                wv_sb.append(t)
            bqk_sb = wpool.tile([128, 2], F32, tag="bqk")
            nc.scalar.dma_start(out=bqk_sb[:], in_=bqk[:])
            bvb_sb = wpool.tile([128, 256], F32, tag="bvb")
            bvb_ap = bvb[:]
            nc.scalar.dma_start(
                out=bvb_sb[:],
                in_=bass.AP(tensor=bvb_ap.tensor, offset=bvb_ap.offset,
                            ap=[[0, 128], [1, 256]]),
            )

            a2a_in = [dram.tile([8, 130, 512], F32R, name=f"a2ain{b}")
                      for b in range(2)]
            a2a_out = [dram.tile([8, 130, 512], F32R, name=f"a2aout{b}")
                       for b in range(2)]

            qT, kT, vS, sums_pre = {}, {}, {}, {}
            last_drain = [None]

            def emit_hsT_load(b):
                tiles = []
                for cc in range(8):
                    t = hpool.tile([128, 2048], F32R, tag=f"hs{cc}",
                                   name=f"hs{b}_{cc}")
                    nc.scalar.dma_start(
                        out=t[:],
                        in_=hsT[128 * cc:128 * (cc + 1), 2048 * b:2048 * (b + 1)])
                    tiles.append(t)
                return tiles

            def emit_proj_qk(b, hs_sb, t_idx, j):
                """One unit: tensor t_idx (0=q,1=k), one 512-wide s-slice j."""
                if t_idx == 0:
                    if b not in qT:
                        qT[b] = qkpool.tile([128, 2048], F32R, tag="qT",
                                            name=f"qT{b}")
                    dst, w_sb = qT[b], wq_sb
                else:
                    if b not in kT:
                        kT[b] = qkpool.tile([128, 2048], F32R, tag="kT",
                                            name=f"kT{b}")
                    dst, w_sb = kT[b], wk_sb
                ps = psum.tile([128, 512], F32, tag="big", bufs=3,
                               name=f"pqk{b}_{t_idx}_{j}")
                for cc in range(8):
                    nc.tensor.matmul(
                        ps[:], w_sb[cc][:],
                        hs_sb[cc][:, 512 * j:512 * (j + 1)],
                        start=(cc == 0), stop=(cc == 7))
                nc.vector.tensor_scalar_add(
                    out=dst[:, 512 * j:512 * (j + 1)], in0=ps[:],
                    scalar1=bqk_sb[:, t_idx:t_idx + 1])

            def emit_proj_v(b, hs_sb, i):
                """One unit: one 128-row v' s-tile i."""
                if b not in vS:
                    vS[b] = qkpool.tile([128, 2080], F32R, tag="vS",
                                        name=f"vS{b}")
                dst = vS[b]
                ps = psum.tile([128, 512], F32, tag="big", bufs=3,
                               name=f"pv{b}_{i}")
                sl = ps[:, 0:256]
                for cc in range(8):
                    nc.tensor.matmul(
                        sl, hs_sb[cc][:, 128 * i:128 * (i + 1)], wv_sb[cc][:],
                        start=(cc == 0), stop=(cc == 7))
                nc.vector.tensor_tensor(
                    out=dst[:, 130 * i:130 * (i + 1)], in0=sl[:, 0:130],
                    in1=bvb_sb[:, 0:130], op=mybir.AluOpType.add)

            def emit_attention_qs(b, qs, fill_work, fill_at):
                """One q-slice (512 q) for both heads, software-pipelined:
                QK(kc+1) is emitted before PV(kc) so ACT paces the loop."""
                accA = psum.tile([65, 512], F32, tag="accA", bufs=1,
                                 name=f"accA_{b}_{qs}")
                accB = psum.tile([65, 512], F32, tag="accB", bufs=1,
                                 name=f"accB_{b}_{qs}")
                sc_t = {}

                def emit_qk(kc):
                    sc = psum.tile([128, 1024], F32, tag="big", bufs=3,
                                   name=f"sc_{b}_{qs}_{kc}")
                    sc_t[kc] = sc
                    nc.tensor.matmul(
                        sc[:, 0:512],
                        kT[b][0:64, 128 * kc:128 * (kc + 1)],
                        qT[b][0:64, 512 * qs:512 * (qs + 1)],
                        start=True, stop=True, tile_position=(0, 0))
                    nc.tensor.matmul(
                        sc[:, 512:1024],
                        kT[b][64:128, 128 * kc:128 * (kc + 1)],
                        qT[b][64:128, 512 * qs:512 * (qs + 1)],
                        start=True, stop=True, tile_position=(64, 0))

                emit_qk(0)
                for kc in range(16):
                    pr = ppool.tile([128, 1024], F32R, tag="pr",
                                    name=f"pr_{b}_{qs}_{kc}")
                    nc.scalar.activation(pr[:], sc_t.pop(kc)[:],
                                         mybir.ActivationFunctionType.Exp,
                                         scale=float(SCALE))
                    if kc < 15:
                        emit_qk(kc + 1)
                    nc.tensor.matmul(
                        accA[:],
                        vS[b][:, 130 * kc + 0:130 * kc + 65],
                        pr[:, 0:512],
                        start=(kc == 0), stop=(kc == 15))
                    nc.tensor.matmul(
                        accB[:],
                        vS[b][:, 130 * kc + 65:130 * kc + 130],
                        pr[:, 512:1024],
                        start=(kc == 0), stop=(kc == 15))
                    if kc in fill_at and fill_work:
                        fill_work.pop(0)()
                # drain: rows [0:64] -> a2a_in, row 64 (sums) -> sums_pre
                if b not in sums_pre:
                    sums_pre[b] = opool.tile([8, 512], F32R, tag="sums",
                                             name=f"sums{b}")
                j = 4 * b + qs
                for h, acc in ((0, accA), (1, accB)):
                    st = spool.tile([65, 512], F32R, tag="st",
                                    name=f"st_{b}_{qs}_{h}")
                    nc.vector.tensor_copy(st[:], acc[:])
                    d = nc.sync.dma_start(
                        out=a2a_in[b][j, 64 * h:64 * (h + 1), :],
                        in_=st[0:64, :])
                    last_drain[0] = d
                    nc.sync.dma_start(
                        out=sums_pre[b][2 * qs + h:2 * qs + h + 1, :],
                        in_=st[64:65, :])

            def emit_recip_ship(b):
                with nc.allow_low_precision("f32r softmax denominators"):
                    nc.vector.reciprocal(sums_pre[b][:],
                                         sums_pre[b][:].bitcast(F32))
                for qs in range(4):
                    j = 4 * b + qs
                    for h in range(2):
                        nc.sync.dma_start(
                            out=a2a_in[b][j, 128 + h:129 + h, :],
                            in_=sums_pre[b][2 * qs + h:2 * qs + h + 1, :])

            def emit_collective(b):
                nc.gpsimd.collective_compute(
                    "AllToAll", mybir.AluOpType.bypass,
                    replica_groups=[list(range(8))],
                    ins=[a2a_in[b][:]], outs=[a2a_out[b][:]])

            def emit_output(b, out_t, res_sb, after=None):
                """Normalize received chunks and run outproj, per chunk."""
                op_ps = []
                for st_i in range(4):
                    if st_i < 3:
                        ps = psum.tile([128, 1024], F32, tag="big", bufs=3,
                                       name=f"op{b}_{st_i}")
                        op_ps.append((ps[:, 0:512], ps[:, 512:1024], ps))
                    else:
                        pa = psum.tile([128, 512], F32, tag="accA", bufs=1,
                                       name=f"op{b}_3a")
                        pb = psum.tile([128, 512], F32, tag="accB", bufs=1,
                                       name=f"op{b}_3b")
                        op_ps.append((pa[:], pb[:], None))
                for j in range(8):
                    raw = opool.tile([128, 512], F32, tag="raw",
                                     name=f"raw{b}_{j}")
                    rd = nc.sync.dma_start(out=raw[:],
                                           in_=a2a_out[b][j, 0:128, :]
                                           .bitcast(F32))
                    if after is not None and j == 0:
                        tile_rust.add_dep_helper(
                            rd.ins, after.ins, False,
                            "hold output norm until attention drained")
                    rbc = opool.tile([128, 512], F32, tag="rbc",
                                     name=f"rbc{b}_{j}")
                    for h in range(2):
                        srow = a2a_out[b][j, 128 + h:129 + h, :].bitcast(F32)
                        nc.sync.dma_start(
                            out=rbc[64 * h:64 * (h + 1), :],
                            in_=bass.AP(tensor=srow.tensor, offset=srow.offset,
                                        ap=[[0, 64], [1, 512]]))
                    an_t = opool.tile([128, 512], F32R, tag="an",
                                      name=f"an{b}_{j}")
                    an = an_t[:]
                    nc.vector.tensor_tensor(out=an, in0=raw[:], in1=rbc[:],
                                            op=mybir.AluOpType.mult)
                    for st_i in range(4):
                        for co in range(2):
                            nc.tensor.matmul(
                                op_ps[st_i][co],
                                an[:, 128 * st_i:128 * (st_i + 1)],
                                wo_sb[j][:, 512 * co:512 * (co + 1)],
                                start=(j == 0), stop=(j == 7))
                for st_i in range(4):
                    ob = opool.tile([128, 1024], F32, tag="ob",
                                    name=f"ob{b}_{st_i}")
                    if st_i < 3:
                        nc.vector.tensor_tensor(out=ob[:],
                                                in0=op_ps[st_i][2][:],
                                                in1=res_sb[st_i][:],
                                                op=mybir.AluOpType.add)
                    else:
                        for co in range(2):
                            nc.vector.tensor_tensor(
                                out=ob[:, 512 * co:512 * (co + 1)],
                                in0=op_ps[st_i][co],
                                in1=res_sb[st_i][:, 512 * co:512 * (co + 1)],
                                op=mybir.AluOpType.add)
                    nc.sync.dma_start(
                        out=out_t[128 * st_i:128 * (st_i + 1), :], in_=ob[:])

            # ---------------- emission ----------------
            # prefix: just enough b0 projection for attention(b0, qs0) kc 0-3
            emit_proj_qk(0, hs0, 0, 0)
            emit_proj_qk(0, hs0, 1, 0)
            for i in range(4):
                emit_proj_v(0, hs0, i)

            hs1 = emit_hsT_load(1)

            def qk_u(b, hs, t, j):
                return lambda: emit_proj_qk(b, hs, t, j)

            def v_u(b, hs, i):
                return lambda: emit_proj_v(b, hs, i)

            # qs0 fill: each unit popped before its first consumer
            # (vS stile i needed at PV(kc=i); kT unit j at QK(kc=4j), which is
            # emitted during iteration kc=4j-1)
            fill = [v_u(0, hs0, 4), v_u(0, hs0, 5), qk_u(0, hs0, 1, 1),
                    v_u(0, hs0, 6), v_u(0, hs0, 7), v_u(0, hs0, 8),
                    qk_u(0, hs0, 1, 2), v_u(0, hs0, 9), v_u(0, hs0, 10),
                    v_u(0, hs0, 11), qk_u(0, hs0, 1, 3), v_u(0, hs0, 12),
                    v_u(0, hs0, 13), v_u(0, hs0, 14), v_u(0, hs0, 15),
                    qk_u(0, hs0, 0, 1)]
            emit_attention_qs(0, 0, fill, tuple(range(16)))
            fill = [qk_u(0, hs0, 0, 2), qk_u(0, hs0, 0, 3)]
            emit_attention_qs(0, 1, fill, (0, 8))
            fill = []
            for t_idx in range(2):
                for j in range(4):
                    fill.append(qk_u(1, hs1, t_idx, j))
            for i in range(16):
                fill.append(v_u(1, hs1, i))
            emit_attention_qs(0, 2, fill, tuple(range(16)))
            emit_attention_qs(0, 3, fill, tuple(range(16)))
            while fill:
                fill.pop(0)()
            emit_recip_ship(0)
            emit_collective(0)

            # load wo / res during attention(b1); reuse freed slots
            for cc in range(8):
                t = hpool.tile([128, 1024], F32R, tag=f"hs{cc}",
                               name=f"wo{cc}")
                nc.sync.dma_start(out=t[:], in_=wo[128 * cc:128 * (cc + 1), :])
                wo_sb.append(t)
            res_sb = []
            for st_i in range(4):
                t = wpool.tile([128, 1024], F32, tag=f"res{st_i}",
                               name=f"res{st_i}")
                nc.sync.dma_start(out=t[:],
                                  in_=res[128 * st_i:128 * (st_i + 1), :])
                res_sb.append(t)

            for qs in range(4):
                emit_attention_qs(1, qs, [], ())
            emit_recip_ship(1)
            emit_output(0, out1, res_sb, after=last_drain[0])
            emit_collective(1)
            emit_output(1, out2, res_sb)
    nc.finalize()
    return nc


def _prep_inputs(hidden_states, Wq, bq, Wk, bk, Wv, bv, Wo, bo):
    hs = np.asarray(hidden_states, np.float32)
    hsT = np.ascontiguousarray(
        hs.transpose(2, 0, 1).reshape(C, BS)).astype(np.float32)
    Wo_f = np.ascontiguousarray(np.asarray(Wo, np.float32))
    in_maps = []
    for c in range(N_CORES):
        h0 = 2 * c
        cols = slice(64 * h0, 64 * h0 + 128)
        wv_c = np.zeros((C, 256), np.float32)
        bvb_c = np.zeros((1, 256), np.float32)
        for a in range(2):
            hd = slice(64 * (h0 + a), 64 * (h0 + a + 1))
            wv_c[:, 65 * a:65 * a + 64] = np.asarray(Wv, np.float32)[:, hd]
            bvb_c[0, 65 * a:65 * a + 64] = np.asarray(bv, np.float32)[hd]
            bvb_c[0, 65 * a + 64] = 1.0
        bqk_c = np.stack([np.asarray(bq, np.float32)[cols],
                          np.asarray(bk, np.float32)[cols]], axis=1)
        b_c, s0 = c // 4, 512 * (c % 4)
        res_c = (hs[b_c, s0:s0 + 512, :] + np.asarray(bo, np.float32)
                 ).astype(np.float32)
        in_maps.append({
            "hsT": hsT,
            "wq": np.ascontiguousarray(np.asarray(Wq, np.float32)[:, cols]),
            "wk": np.ascontiguousarray(np.asarray(Wk, np.float32)[:, cols]),
            "wv": wv_c,
            "wo": Wo_f,
            "bqk": np.ascontiguousarray(bqk_c),
            "bvb": bvb_c,
            "res": np.ascontiguousarray(res_c),
        })
    return in_maps


def _run(inputs, trace=False, trace_kwargs=None):
    if "nc" not in _CACHE:
        _CACHE["nc"] = _build()
    nc = _CACHE["nc"]
    in_maps = _prep_inputs(**inputs)
    r = run_bass_kernel_spmd(nc, in_maps, core_ids=list(range(N_CORES)),
                             trace=trace, **(trace_kwargs or {}))
    full = np.empty((B, S, C), np.float32)
    for c in range(N_CORES):
        key = "out1" if c < 4 else "out2"
        full[c // 4, 512 * (c % 4):512 * (c % 4 + 1), :] = r.results[c][key]
    return full, r


def kernel(**inputs):
    full, _ = _run(inputs, trace=False)
    return full


# revision 15
# speedup vs baseline: 1.0631x; 1.0631x over previous
"""Trainium2 Bass kernel for AttnProcessor self-attention (B=2,S=2048,C=1024,H=16).

Sharding: 8 cores, core c owns heads (2c, 2c+1) for both batches (tensor
parallel on the head dim for QKV); attention outputs are redistributed with
two 8-core AllToAlls (one per batch, the first hidden under batch-1 compute)
so core c computes the output projection + residual for output rows
(b=c//4, s in [512*(c%4), 512*(c%4+1))). Host picks out1/out2 per core.

Per-core pipeline (all matmuls fp32r):
  qT/kT projections in [c'=128, s] layout, v' in [s, 2x(64+ones)] layout
  (padded to 256 free for fp32r full rate), row-tiled (64x128) QK^T per head
  pair, exp on ScalarE (scale=1/8, no max subtraction -- scores are O(5)),
  PV accumulates V'.T @ probsT giving both the attention output (transposed)
  and the softmax denominators (ones row). Denominator reciprocals are
  computed batched pre-collective and ride the AllToAll; normalization +
  output projection run per received chunk.
"""
import numpy as np

import concourse.bacc as bacc
import concourse.bass as bass
import concourse.tile as tile
import concourse.tile_rust as tile_rust
from concourse import mybir
from concourse.bass_utils import run_bass_kernel_spmd

F32 = mybir.dt.float32
F32R = mybir.dt.float32r

B, S, C, H, D = 2, 2048, 1024, 16, 64
N_CORES = 8
BS = B * S  # 4096
SCALE = 1.0 / np.sqrt(D)

_CACHE = {}


def _build():
    nc = bacc.Bacc(num_devices=N_CORES)
    hsT = nc.declare_dram_parameter("hsT", [C, BS], F32R, isOutput=False)
    wq = nc.declare_dram_parameter("wq", [C, 128], F32R, isOutput=False)
    wk = nc.declare_dram_parameter("wk", [C, 128], F32R, isOutput=False)
    wv = nc.declare_dram_parameter("wv", [C, 256], F32R, isOutput=False)
    wo = nc.declare_dram_parameter("wo", [C, C], F32R, isOutput=False)
    bqk = nc.declare_dram_parameter("bqk", [128, 2], F32, isOutput=False)
    bvb = nc.declare_dram_parameter("bvb", [1, 256], F32, isOutput=False)
    res = nc.declare_dram_parameter("res", [512, C], F32, isOutput=False)
    out1 = nc.declare_dram_parameter("out1", [512, C], F32, isOutput=True)
    out2 = nc.declare_dram_parameter("out2", [512, C], F32, isOutput=True)

    with tile.TileContext(nc) as tc:
        with (
            tc.tile_pool(name="wpool", bufs=1) as wpool,
            tc.tile_pool(name="hpool", bufs=1) as hpool,
            tc.tile_pool(name="qkpool", bufs=2) as qkpool,
            tc.tile_pool(name="ppool", bufs=3) as ppool,
            tc.tile_pool(name="spool", bufs=3) as spool,
            tc.tile_pool(name="opool", bufs=2) as opool,
            tc.tile_pool(name="psum", bufs=1, space="PSUM") as psum,
            tc.tile_pool(name="dram", bufs=1, space="DRAM") as dram,
        ):
            # ---- weight / constant / input loads (hsT early: long pole) ----
            wq_sb, wk_sb, wv_sb, wo_sb = [], [], [], []
            for cc in range(8):
                t = wpool.tile([128, 128], F32R, tag=f"wq{cc}")
                nc.scalar.dma_start(out=t[:], in_=wq[128 * cc:128 * (cc + 1), :])
                wq_sb.append(t)
            hs0 = []
            for cc in range(8):
                t = hpool.tile([128, 2048], F32R, tag=f"hs{cc}", name=f"hs0_{cc}")
                hs0.append(t)
            for g in range(4):
                for cc in range(8):
                    nc.scalar.dma_start(
                        out=hs0[cc][:, 512 * g:512 * (g + 1)],
                        in_=hsT[128 * cc:128 * (cc + 1),
                                512 * g:512 * (g + 1)])
            for cc in range(8):
                t = wpool.tile([128, 128], F32R, tag=f"wk{cc}")
                nc.scalar.dma_start(out=t[:], in_=wk[128 * cc:128 * (cc + 1), :])
                wk_sb.append(t)
                t = wpool.tile([128, 256], F32R, tag=f"wv{cc}")
                nc.scalar.dma_start(out=t[:], in_=wv[128 * cc:128 * (cc + 1), :])
                wv_sb.append(t)
            bqk_sb = wpool.tile([128, 2], F32, tag="bqk")
            nc.scalar.dma_start(out=bqk_sb[:], in_=bqk[:])
            bvb_sb = wpool.tile([128, 256], F32, tag="bvb")
            bvb_ap = bvb[:]
            nc.scalar.dma_start(
                out=bvb_sb[:],
                in_=bass.AP(tensor=bvb_ap.tensor, offset=bvb_ap.offset,
                            ap=[[0, 128], [1, 256]]),
            )

            a2a_in = [dram.tile([8, 130, 512], F32R, name=f"a2ain{b}")
                      for b in range(2)]
            a2a_out = [dram.tile([8, 130, 512], F32R, name=f"a2aout{b}")
                       for b in range(2)]

            qT, kT, vS, sums_pre = {}, {}, {}, {}
            last_drain = [None]

            def emit_hsT_load(b):
                tiles = []
                for cc in range(8):
                    t = hpool.tile([128, 2048], F32R, tag=f"hs{cc}",
                                   name=f"hs{b}_{cc}")
                    nc.scalar.dma_start(
                        out=t[:],
                        in_=hsT[128 * cc:128 * (cc + 1), 2048 * b:2048 * (b + 1)])
                    tiles.append(t)
                return tiles

            def emit_proj_qk(b, hs_sb, t_idx, j):
                """One unit: tensor t_idx (0=q,1=k), one 512-wide s-slice j."""
                if t_idx == 0:
                    if b not in qT:
                        qT[b] = qkpool.tile([128, 2048], F32R, tag="qT",
                                            name=f"qT{b}")
                    dst, w_sb = qT[b], wq_sb
                else:
                    if b not in kT:
                        kT[b] = qkpool.tile([128, 2048], F32R, tag="kT",
                                            name=f"kT{b}")
                    dst, w_sb = kT[b], wk_sb
                ps = psum.tile([128, 512], F32, tag="big", bufs=3,
                               name=f"pqk{b}_{t_idx}_{j}")
                for cc in range(8):
                    nc.tensor.matmul(
                        ps[:], w_sb[cc][:],
                        hs_sb[cc][:, 512 * j:512 * (j + 1)],
                        start=(cc == 0), stop=(cc == 7))
                nc.vector.tensor_scalar_add(
                    out=dst[:, 512 * j:512 * (j + 1)], in0=ps[:],
                    scalar1=bqk_sb[:, t_idx:t_idx + 1])

            def emit_proj_v(b, hs_sb, i):
                """One unit: one 128-row v' s-tile i."""
                if b not in vS:
                    vS[b] = qkpool.tile([128, 2080], F32R, tag="vS",
                                        name=f"vS{b}")
                dst = vS[b]
                ps = psum.tile([128, 512], F32, tag="big", bufs=3,
                               name=f"pv{b}_{i}")
                sl = ps[:, 0:256]
                for cc in range(8):
                    nc.tensor.matmul(
                        sl, hs_sb[cc][:, 128 * i:128 * (i + 1)], wv_sb[cc][:],
                        start=(cc == 0), stop=(cc == 7))
                nc.vector.tensor_tensor(
                    out=dst[:, 130 * i:130 * (i + 1)], in0=sl[:, 0:130],
                    in1=bvb_sb[:, 0:130], op=mybir.AluOpType.add)

            def emit_attention_qs(b, qs, fill_work):
                """One q-slice (512 q) for both heads, processed in kc-pairs:
                per step, fills then 2 exps, then 4 QK mms (64-row config),
                then 4 PV mms (128-row config, bank-paired A,A,B,B)."""
                accA = psum.tile([65, 512], F32, tag="accA", bufs=1,
                                 name=f"accA_{b}_{qs}")
                accB = psum.tile([65, 512], F32, tag="accB", bufs=1,
                                 name=f"accB_{b}_{qs}")
                sc_t = {}

                def emit_qk(kc):
                    sc = psum.tile([128, 1024], F32, tag="big", bufs=3,
                                   name=f"sc_{b}_{qs}_{kc}")
                    sc_t[kc] = sc
                    nc.tensor.matmul(
                        sc[:, 0:512],
                        kT[b][0:64, 128 * kc:128 * (kc + 1)],
                        qT[b][0:64, 512 * qs:512 * (qs + 1)],
                        start=True, stop=True, tile_position=(0, 0))
                    nc.tensor.matmul(
                        sc[:, 512:1024],
                        kT[b][64:128, 128 * kc:128 * (kc + 1)],
                        qT[b][64:128, 512 * qs:512 * (qs + 1)],
                        start=True, stop=True, tile_position=(64, 0))

                def emit_pv(acc, off, kc, pr):
                    nc.tensor.matmul(
                        acc[:],
                        vS[b][:, 130 * kc + off:130 * kc + off + 65],
                        pr[:, (0 if off == 0 else 512):
                           (512 if off == 0 else 1024)],
                        start=(kc == 0), stop=(kc == 15))

                emit_qk(0)
                emit_qk(1)
                for step in range(8):
                    kc0, kc1 = 2 * step, 2 * step + 1
                    for _ in range(2):
                        if fill_work:
                            fill_work.pop(0)()
                    pr0 = ppool.tile([128, 1024], F32R, tag="pr",
                                     name=f"pr_{b}_{qs}_{kc0}")
                    nc.scalar.activation(pr0[:], sc_t.pop(kc0)[:],
                                         mybir.ActivationFunctionType.Exp,
                                         scale=float(SCALE))
                    pr1 = ppool.tile([128, 1024], F32R, tag="pr",
                                     name=f"pr_{b}_{qs}_{kc1}")
                    nc.scalar.activation(pr1[:], sc_t.pop(kc1)[:],
                                         mybir.ActivationFunctionType.Exp,
                                         scale=float(SCALE))
                    if step < 7:
                        emit_qk(kc0 + 2)
                        emit_qk(kc1 + 2)
                    emit_pv(accA, 0, kc0, pr0)
                    emit_pv(accA, 0, kc1, pr1)
                    emit_pv(accB, 65, kc0, pr0)
                    emit_pv(accB, 65, kc1, pr1)
                # drain: rows [0:64] -> a2a_in, row 64 (sums) -> sums_pre
                if b not in sums_pre:
                    sums_pre[b] = opool.tile([8, 512], F32R, tag="sums",
                                             name=f"sums{b}")
                j = 4 * b + qs
                for h, acc in ((0, accA), (1, accB)):
                    st = spool.tile([65, 512], F32R, tag="st",
                                    name=f"st_{b}_{qs}_{h}")
                    nc.vector.tensor_copy(st[:], acc[:])
                    d = nc.sync.dma_start(
                        out=a2a_in[b][j, 64 * h:64 * (h + 1), :],
                        in_=st[0:64, :])
                    last_drain[0] = d
                    nc.sync.dma_start(
                        out=sums_pre[b][2 * qs + h:2 * qs + h + 1, :],
                        in_=st[64:65, :])

            def emit_recip_ship(b):
                with nc.allow_low_precision("f32r softmax denominators"):
                    nc.vector.reciprocal(sums_pre[b][:],
                                         sums_pre[b][:].bitcast(F32))
                for qs in range(4):
                    j = 4 * b + qs
                    for h in range(2):
                        nc.sync.dma_start(
                            out=a2a_in[b][j, 128 + h:129 + h, :],
                            in_=sums_pre[b][2 * qs + h:2 * qs + h + 1, :])

            def emit_collective(b):
                nc.gpsimd.collective_compute(
                    "AllToAll", mybir.AluOpType.bypass,
                    replica_groups=[list(range(8))],
                    ins=[a2a_in[b][:]], outs=[a2a_out[b][:]])

            def emit_output(b, out_t, res_sb, after=None):
                """Normalize received chunks and run outproj, per chunk."""
                op_ps = []
                for st_i in range(4):
                    if st_i < 3:
                        ps = psum.tile([128, 1024], F32, tag="big", bufs=3,
                                       name=f"op{b}_{st_i}")
                        op_ps.append((ps[:, 0:512], ps[:, 512:1024], ps))
                    else:
                        pa = psum.tile([128, 512], F32, tag="accA", bufs=1,
                                       name=f"op{b}_3a")
                        pb = psum.tile([128, 512], F32, tag="accB", bufs=1,
                                       name=f"op{b}_3b")
                        op_ps.append((pa[:], pb[:], None))
                for j in range(8):
                    raw = opool.tile([128, 512], F32, tag="raw",
                                     name=f"raw{b}_{j}")
                    rd = nc.sync.dma_start(out=raw[:],
                                           in_=a2a_out[b][j, 0:128, :]
                                           .bitcast(F32))
                    if after is not None and j == 0:
                        tile_rust.add_dep_helper(
                            rd.ins, after.ins, False,
                            "hold output norm until attention drained")
                    rbc = opool.tile([128, 512], F32, tag="rbc",
                                     name=f"rbc{b}_{j}")
                    for h in range(2):
                        srow = a2a_out[b][j, 128 + h:129 + h, :].bitcast(F32)
                        nc.sync.dma_start(
                            out=rbc[64 * h:64 * (h + 1), :],
                            in_=bass.AP(tensor=srow.tensor, offset=srow.offset,
                                        ap=[[0, 64], [1, 512]]))
                    an_t = opool.tile([128, 512], F32R, tag="an",
                                      name=f"an{b}_{j}")
                    an = an_t[:]
                    nc.vector.tensor_tensor(out=an, in0=raw[:], in1=rbc[:],
                                            op=mybir.AluOpType.mult)
                    for st_i in range(4):
                        for co in range(2):
                            nc.tensor.matmul(
                                op_ps[st_i][co],
                                an[:, 128 * st_i:128 * (st_i + 1)],
                                wo_sb[j][:, 512 * co:512 * (co + 1)],
                                start=(j == 0), stop=(j == 7))
                for st_i in range(4):
                    ob = opool.tile([128, 1024], F32, tag="ob",
                                    name=f"ob{b}_{st_i}")
                    if st_i < 3:
                        nc.vector.tensor_tensor(out=ob[:],
                                                in0=op_ps[st_i][2][:],
                                                in1=res_sb[st_i][:],
                                                op=mybir.AluOpType.add)
                    else:
                        for co in range(2):
                            nc.vector.tensor_tensor(
                                out=ob[:, 512 * co:512 * (co + 1)],
                                in0=op_ps[st_i][co],
                                in1=res_sb[st_i][:, 512 * co:512 * (co + 1)],
                                op=mybir.AluOpType.add)
                    nc.sync.dma_start(
                        out=out_t[128 * st_i:128 * (st_i + 1), :], in_=ob[:])

            # ---------------- emission ----------------
            # prefix: just enough b0 projection for attention(b0, qs0) kc 0-3
            emit_proj_qk(0, hs0, 0, 0)
            emit_proj_qk(0, hs0, 1, 0)
            for i in range(4):
                emit_proj_v(0, hs0, i)

            hs1 = emit_hsT_load(1)

            def qk_u(b, hs, t, j):
                return lambda: emit_proj_qk(b, hs, t, j)

            def v_u(b, hs, i):
                return lambda: emit_proj_v(b, hs, i)

            # qs0 fill: 2 pops per step-start; each unit lands before its
            # first consumer (vS stile i -> PV at step i//2; kT unit j ->
            # QK(4j) emitted at step 2j-1; deadlines checked offline)
            fill = [qk_u(0, hs0, 1, 1), v_u(0, hs0, 4), v_u(0, hs0, 5),
                    qk_u(0, hs0, 1, 2), v_u(0, hs0, 6), v_u(0, hs0, 7),
                    v_u(0, hs0, 8), v_u(0, hs0, 9), qk_u(0, hs0, 1, 3),
                    v_u(0, hs0, 10), v_u(0, hs0, 11), v_u(0, hs0, 12),
                    v_u(0, hs0, 13), v_u(0, hs0, 14), v_u(0, hs0, 15),
                    qk_u(0, hs0, 0, 1)]
            emit_attention_qs(0, 0, fill)
            fill = [qk_u(0, hs0, 0, 2), qk_u(0, hs0, 0, 3)]
            emit_attention_qs(0, 1, fill)
            fill = []
            for t_idx in range(2):
                for j in range(4):
                    fill.append(qk_u(1, hs1, t_idx, j))
            for i in range(16):
                fill.append(v_u(1, hs1, i))
            emit_attention_qs(0, 2, fill)
            emit_attention_qs(0, 3, fill)
            while fill:
                fill.pop(0)()
            emit_recip_ship(0)
            emit_collective(0)

            # load wo / res during attention(b1); reuse freed slots
            for cc in range(8):
                t = hpool.tile([128, 1024], F32R, tag=f"hs{cc}",
                               name=f"wo{cc}")
                nc.sync.dma_start(out=t[:], in_=wo[128 * cc:128 * (cc + 1), :])
                wo_sb.append(t)
            res_sb = []
            for st_i in range(4):
                t = wpool.tile([128, 1024], F32, tag=f"res{st_i}",
                               name=f"res{st_i}")
                nc.sync.dma_start(out=t[:],
                                  in_=res[128 * st_i:128 * (st_i + 1), :])
                res_sb.append(t)

            for qs in range(4):
                emit_attention_qs(1, qs, [])
            emit_recip_ship(1)
            emit_output(0, out1, res_sb, after=last_drain[0])
            emit_collective(1)
            emit_output(1, out2, res_sb)
    nc.finalize()
    return nc


def _prep_inputs(hidden_states, Wq, bq, Wk, bk, Wv, bv, Wo, bo):
    hs = np.asarray(hidden_states, np.float32)
    hsT = np.ascontiguousarray(
        hs.transpose(2, 0, 1).reshape(C, BS)).astype(np.float32)
    Wo_f = np.ascontiguousarray(np.asarray(Wo, np.float32))
    in_maps = []
    for c in range(N_CORES):
        h0 = 2 * c
        cols = slice(64 * h0, 64 * h0 + 128)
        wv_c = np.zeros((C, 256), np.float32)
        bvb_c = np.zeros((1, 256), np.float32)
        for a in range(2):
            hd = slice(64 * (h0 + a), 64 * (h0 + a + 1))
            wv_c[:, 65 * a:65 * a + 64] = np.asarray(Wv, np.float32)[:, hd]
            bvb_c[0, 65 * a:65 * a + 64] = np.asarray(bv, np.float32)[hd]
            bvb_c[0, 65 * a + 64] = 1.0
        bqk_c = np.stack([np.asarray(bq, np.float32)[cols],
                          np.asarray(bk, np.float32)[cols]], axis=1)
        b_c, s0 = c // 4, 512 * (c % 4)
        res_c = (hs[b_c, s0:s0 + 512, :] + np.asarray(bo, np.float32)
                 ).astype(np.float32)
        in_maps.append({
            "hsT": hsT,
            "wq": np.ascontiguousarray(np.asarray(Wq, np.float32)[:, cols]),
            "wk": np.ascontiguousarray(np.asarray(Wk, np.float32)[:, cols]),
            "wv": wv_c,
            "wo": Wo_f,
            "bqk": np.ascontiguousarray(bqk_c),
            "bvb": bvb_c,
            "res": np.ascontiguousarray(res_c),
        })
    return in_maps


def _run(inputs, trace=False, trace_kwargs=None):
    if "nc" not in _CACHE:
        _CACHE["nc"] = _build()
    nc = _CACHE["nc"]
    in_maps = _prep_inputs(**inputs)
    r = run_bass_kernel_spmd(nc, in_maps, core_ids=list(range(N_CORES)),
                             trace=trace, **(trace_kwargs or {}))
    full = np.empty((B, S, C), np.float32)
    for c in range(N_CORES):
        key = "out1" if c < 4 else "out2"
        full[c // 4, 512 * (c % 4):512 * (c % 4 + 1), :] = r.results[c][key]
    return full, r


def kernel(**inputs):
    full, _ = _run(inputs, trace=False)
    return full


# revision 16
# speedup vs baseline: 1.1253x; 1.0586x over previous
"""Trainium2 Bass kernel for AttnProcessor self-attention (B=2,S=2048,C=1024,H=16).

Sharding: 8 cores, core c owns heads (2c, 2c+1) for both batches (tensor
parallel on the head dim for QKV); attention outputs are redistributed with
two 8-core AllToAlls (one per batch, the first hidden under batch-1 compute)
so core c computes the output projection + residual for output rows
(b=c//4, s in [512*(c%4), 512*(c%4+1))). Host picks out1/out2 per core.

Per-core pipeline (all matmuls fp32r):
  qT/kT projections in [c'=128, s] layout, v' in [s, 2x(64+ones)] layout
  (padded to 256 free for fp32r full rate), row-tiled (64x128) QK^T per head
  pair, exp on ScalarE (scale=1/8, no max subtraction -- scores are O(5)),
  PV accumulates V'.T @ probsT giving both the attention output (transposed)
  and the softmax denominators (ones row). Denominator reciprocals are
  computed batched pre-collective and ride the AllToAll; normalization +
  output projection run per received chunk.
"""
import numpy as np

import concourse.bacc as bacc
import concourse.bass as bass
import concourse.tile as tile
import concourse.tile_rust as tile_rust
from concourse import mybir
from concourse.bass_utils import run_bass_kernel_spmd

F32 = mybir.dt.float32
F32R = mybir.dt.float32r

B, S, C, H, D = 2, 2048, 1024, 16, 64
N_CORES = 8
BS = B * S  # 4096
SCALE = 1.0 / np.sqrt(D)

_CACHE = {}


def _build():
    nc = bacc.Bacc(num_devices=N_CORES)
    hsT = nc.declare_dram_parameter("hsT", [C, BS], F32R, isOutput=False)
    wq = nc.declare_dram_parameter("wq", [C, 128], F32R, isOutput=False)
    wk = nc.declare_dram_parameter("wk", [C, 128], F32R, isOutput=False)
    wv = nc.declare_dram_parameter("wv", [C, 256], F32R, isOutput=False)
    wo = nc.declare_dram_parameter("wo", [C, C], F32R, isOutput=False)
    bqk = nc.declare_dram_parameter("bqk", [128, 2], F32, isOutput=False)
    bvb = nc.declare_dram_parameter("bvb", [1, 256], F32, isOutput=False)
    res = nc.declare_dram_parameter("res", [512, C], F32, isOutput=False)
    out1 = nc.declare_dram_parameter("out1", [512, C], F32, isOutput=True)
    out2 = nc.declare_dram_parameter("out2", [512, C], F32, isOutput=True)

    with tile.TileContext(nc) as tc:
        with (
            tc.tile_pool(name="wpool", bufs=1) as wpool,
            tc.tile_pool(name="hpool", bufs=1) as hpool,
            tc.tile_pool(name="qkpool", bufs=2) as qkpool,
            tc.tile_pool(name="ppool", bufs=3) as ppool,
            tc.tile_pool(name="spool", bufs=3) as spool,
            tc.tile_pool(name="opool", bufs=2) as opool,
            tc.tile_pool(name="psum", bufs=1, space="PSUM") as psum,
            tc.tile_pool(name="dram", bufs=1, space="DRAM") as dram,
        ):
            # ---- weight / constant / input loads ----
            # single strided DMA per weight tensor: sbuf [128, 8*N] with
            # chunk cc at cols N*cc  <-  dram [1024, N]
            wo_sb = []

            def load_w(name, src, ncols):
                t = wpool.tile([128, 8 * ncols], F32R, tag=name)
                sap = src[:]
                nc.scalar.dma_start(
                    out=t[:],
                    in_=bass.AP(tensor=sap.tensor, offset=sap.offset,
                                ap=[[ncols, 128], [128 * ncols, 8],
                                    [1, ncols]]))
                return [t[:, ncols * cc:ncols * (cc + 1)] for cc in range(8)]

            wq_sb = load_w("wq", wq, 128)
            hs0 = []
            for cc in range(8):
                t = hpool.tile([128, 2048], F32R, tag=f"hs{cc}", name=f"hs0_{cc}")
                hs0.append(t)
            for g in range(2):
                for cc in range(8):
                    eng = nc.scalar if g == 0 else nc.sync
                    eng.dma_start(
                        out=hs0[cc][:, 1024 * g:1024 * (g + 1)],
                        in_=hsT[128 * cc:128 * (cc + 1),
                                1024 * g:1024 * (g + 1)])
            wk_sb = load_w("wk", wk, 128)
            wv_sb = load_w("wv", wv, 256)
            bqk_sb = wpool.tile([128, 2], F32, tag="bqk")
            nc.scalar.dma_start(out=bqk_sb[:], in_=bqk[:])
            bvb_sb = wpool.tile([128, 256], F32, tag="bvb")
            bvb_ap = bvb[:]
            nc.scalar.dma_start(
                out=bvb_sb[:],
                in_=bass.AP(tensor=bvb_ap.tensor, offset=bvb_ap.offset,
                            ap=[[0, 128], [1, 256]]),
            )

            a2a_in = [dram.tile([8, 130, 512], F32R, name=f"a2ain{b}")
                      for b in range(2)]
            a2a_out = [dram.tile([8, 130, 512], F32R, name=f"a2aout{b}")
                       for b in range(2)]

            qT, kT, vS, sums_pre = {}, {}, {}, {}
            last_drain = [None]

            def emit_hsT_load(b):
                tiles = []
                for cc in range(8):
                    t = hpool.tile([128, 2048], F32R, tag=f"hs{cc}",
                                   name=f"hs{b}_{cc}")
                    nc.scalar.dma_start(
                        out=t[:],
                        in_=hsT[128 * cc:128 * (cc + 1), 2048 * b:2048 * (b + 1)])
                    tiles.append(t)
                return tiles

            def emit_proj_qk(b, hs_sb, t_idx, j):
                """One unit: tensor t_idx (0=q,1=k), one 512-wide s-slice j."""
                if t_idx == 0:
                    if b not in qT:
                        qT[b] = qkpool.tile([128, 2048], F32R, tag="qT",
                                            name=f"qT{b}")
                    dst, w_sb = qT[b], wq_sb
                else:
                    if b not in kT:
                        kT[b] = qkpool.tile([128, 2048], F32R, tag="kT",
                                            name=f"kT{b}")
                    dst, w_sb = kT[b], wk_sb
                ps = psum.tile([128, 512], F32, tag="big", bufs=3,
                               name=f"pqk{b}_{t_idx}_{j}")
                for cc in range(8):
                    nc.tensor.matmul(
                        ps[:], w_sb[cc],
                        hs_sb[cc][:, 512 * j:512 * (j + 1)],
                        start=(cc == 0), stop=(cc == 7))
                nc.vector.tensor_scalar_add(
                    out=dst[:, 512 * j:512 * (j + 1)], in0=ps[:],
                    scalar1=bqk_sb[:, t_idx:t_idx + 1])

            def emit_proj_v(b, hs_sb, i):
                """One unit: one 128-row v' s-tile i."""
                if b not in vS:
                    vS[b] = qkpool.tile([128, 2080], F32R, tag="vS",
                                        name=f"vS{b}")
                dst = vS[b]
                ps = psum.tile([128, 512], F32, tag="big", bufs=3,
                               name=f"pv{b}_{i}")
                sl = ps[:, 0:256]
                for cc in range(8):
                    nc.tensor.matmul(
                        sl, hs_sb[cc][:, 128 * i:128 * (i + 1)], wv_sb[cc],
                        start=(cc == 0), stop=(cc == 7))
                nc.vector.tensor_tensor(
                    out=dst[:, 130 * i:130 * (i + 1)], in0=sl[:, 0:130],
                    in1=bvb_sb[:, 0:130], op=mybir.AluOpType.add)

            def emit_attention_qs(b, qs, fill_work):
                """One q-slice (512 q) for both heads, processed in kc-pairs:
                per step, fills then 2 exps, then 4 QK mms (64-row config),
                then 4 PV mms (128-row config, bank-paired A,A,B,B)."""
                accA = psum.tile([65, 512], F32, tag="accA", bufs=1,
                                 name=f"accA_{b}_{qs}")
                accB = psum.tile([65, 512], F32, tag="accB", bufs=1,
                                 name=f"accB_{b}_{qs}")
                sc_t = {}

                def emit_qk(kc):
                    sc = psum.tile([128, 1024], F32, tag="big", bufs=3,
                                   name=f"sc_{b}_{qs}_{kc}")
                    sc_t[kc] = sc
                    nc.tensor.matmul(
                        sc[:, 0:512],
                        kT[b][0:64, 128 * kc:128 * (kc + 1)],
                        qT[b][0:64, 512 * qs:512 * (qs + 1)],
                        start=True, stop=True, tile_position=(0, 0))
                    nc.tensor.matmul(
                        sc[:, 512:1024],
                        kT[b][64:128, 128 * kc:128 * (kc + 1)],
                        qT[b][64:128, 512 * qs:512 * (qs + 1)],
                        start=True, stop=True, tile_position=(64, 0))

                def emit_pv(acc, off, kc, pr):
                    nc.tensor.matmul(
                        acc[:],
                        vS[b][:, 130 * kc + off:130 * kc + off + 65],
                        pr[:, (0 if off == 0 else 512):
                           (512 if off == 0 else 1024)],
                        start=(kc == 0), stop=(kc == 15))

                emit_qk(0)
                emit_qk(1)
                for step in range(8):
                    kc0, kc1 = 2 * step, 2 * step + 1
                    for _ in range(2):
                        if fill_work:
                            fill_work.pop(0)()
                    pr0 = ppool.tile([128, 1024], F32R, tag="pr",
                                     name=f"pr_{b}_{qs}_{kc0}")
                    nc.scalar.activation(pr0[:], sc_t.pop(kc0)[:],
                                         mybir.ActivationFunctionType.Exp,
                                         scale=float(SCALE))
                    pr1 = ppool.tile([128, 1024], F32R, tag="pr",
                                     name=f"pr_{b}_{qs}_{kc1}")
                    nc.scalar.activation(pr1[:], sc_t.pop(kc1)[:],
                                         mybir.ActivationFunctionType.Exp,
                                         scale=float(SCALE))
                    if step < 7:
                        emit_qk(kc0 + 2)
                        emit_qk(kc1 + 2)
                    emit_pv(accA, 0, kc0, pr0)
                    emit_pv(accA, 0, kc1, pr1)
                    emit_pv(accB, 65, kc0, pr0)
                    emit_pv(accB, 65, kc1, pr1)
                # drain: rows [0:64] -> a2a_in, row 64 (sums) -> sums_pre
                if b not in sums_pre:
                    sums_pre[b] = opool.tile([8, 512], F32R, tag="sums",
                                             name=f"sums{b}")
                j = 4 * b + qs
                for h, acc in ((0, accA), (1, accB)):
                    st = spool.tile([65, 512], F32R, tag="st",
                                    name=f"st_{b}_{qs}_{h}")
                    nc.vector.tensor_copy(st[:], acc[:])
                    d = nc.sync.dma_start(
                        out=a2a_in[b][j, 64 * h:64 * (h + 1), :],
                        in_=st[0:64, :])
                    last_drain[0] = d
                    nc.sync.dma_start(
                        out=sums_pre[b][2 * qs + h:2 * qs + h + 1, :],
                        in_=st[64:65, :])

            def emit_recip_ship(b):
                with nc.allow_low_precision("f32r softmax denominators"):
                    nc.vector.reciprocal(sums_pre[b][:],
                                         sums_pre[b][:].bitcast(F32))
                for qs in range(4):
                    j = 4 * b + qs
                    for h in range(2):
                        nc.sync.dma_start(
                            out=a2a_in[b][j, 128 + h:129 + h, :],
                            in_=sums_pre[b][2 * qs + h:2 * qs + h + 1, :])

            def emit_collective(b):
                nc.gpsimd.collective_compute(
                    "AllToAll", mybir.AluOpType.bypass,
                    replica_groups=[list(range(8))],
                    ins=[a2a_in[b][:]], outs=[a2a_out[b][:]])

            def emit_output(b, out_t, res_sb, after=None):
                """Normalize received chunks and run outproj, per chunk."""
                op_ps = []
                for st_i in range(4):
                    if st_i < 3:
                        ps = psum.tile([128, 1024], F32, tag="big", bufs=3,
                                       name=f"op{b}_{st_i}")
                        op_ps.append((ps[:, 0:512], ps[:, 512:1024], ps))
                    else:
                        pa = psum.tile([128, 512], F32, tag="accA", bufs=1,
                                       name=f"op{b}_3a")
                        pb = psum.tile([128, 512], F32, tag="accB", bufs=1,
                                       name=f"op{b}_3b")
                        op_ps.append((pa[:], pb[:], None))
                for j in range(8):
                    raw = opool.tile([128, 512], F32, tag="raw",
                                     name=f"raw{b}_{j}")
                    rd = nc.scalar.dma_start(out=raw[:],
                                             in_=a2a_out[b][j, 0:128, :]
                                             .bitcast(F32))
                    if after is not None and j == 0:
                        tile_rust.add_dep_helper(
                            rd.ins, after.ins, False,
                            "hold output norm until attention drained")
                    rbc = opool.tile([128, 512], F32, tag="rbc",
                                     name=f"rbc{b}_{j}")
                    for h in range(2):
                        srow = a2a_out[b][j, 128 + h:129 + h, :].bitcast(F32)
                        nc.scalar.dma_start(
                            out=rbc[64 * h:64 * (h + 1), :],
                            in_=bass.AP(tensor=srow.tensor, offset=srow.offset,
                                        ap=[[0, 64], [1, 512]]))
                    an_t = opool.tile([128, 512], F32R, tag="an",
                                      name=f"an{b}_{j}")
                    an = an_t[:]
                    nc.vector.tensor_tensor(out=an, in0=raw[:], in1=rbc[:],
                                            op=mybir.AluOpType.mult)
                    for st_i in range(4):
                        for co in range(2):
                            nc.tensor.matmul(
                                op_ps[st_i][co],
                                an[:, 128 * st_i:128 * (st_i + 1)],
                                wo_sb[j][:, 512 * co:512 * (co + 1)],
                                start=(j == 0), stop=(j == 7))
                for st_i in range(4):
                    ob = opool.tile([128, 1024], F32, tag="ob",
                                    name=f"ob{b}_{st_i}")
                    if st_i < 3:
                        nc.vector.tensor_tensor(out=ob[:],
                                                in0=op_ps[st_i][2][:],
                                                in1=res_sb[st_i][:],
                                                op=mybir.AluOpType.add)
                    else:
                        for co in range(2):
                            nc.vector.tensor_tensor(
                                out=ob[:, 512 * co:512 * (co + 1)],
                                in0=op_ps[st_i][co],
                                in1=res_sb[st_i][:, 512 * co:512 * (co + 1)],
                                op=mybir.AluOpType.add)
                    nc.sync.dma_start(
                        out=out_t[128 * st_i:128 * (st_i + 1), :], in_=ob[:])

            # ---------------- emission ----------------
            # prefix: just enough b0 projection for attention(b0, qs0) kc 0-3
            emit_proj_qk(0, hs0, 0, 0)
            emit_proj_qk(0, hs0, 1, 0)
            for i in range(4):
                emit_proj_v(0, hs0, i)

            hs1 = emit_hsT_load(1)

            def qk_u(b, hs, t, j):
                return lambda: emit_proj_qk(b, hs, t, j)

            def v_u(b, hs, i):
                return lambda: emit_proj_v(b, hs, i)

            # qs0 fill: 2 pops per step-start; each unit lands before its
            # first consumer (vS stile i -> PV at step i//2; kT unit j ->
            # QK(4j) emitted at step 2j-1; deadlines checked offline)
            fill = [qk_u(0, hs0, 1, 1), v_u(0, hs0, 4), v_u(0, hs0, 5),
                    qk_u(0, hs0, 1, 2), v_u(0, hs0, 6), v_u(0, hs0, 7),
                    v_u(0, hs0, 8), v_u(0, hs0, 9), qk_u(0, hs0, 1, 3),
                    v_u(0, hs0, 10), v_u(0, hs0, 11), v_u(0, hs0, 12),
                    v_u(0, hs0, 13), v_u(0, hs0, 14), v_u(0, hs0, 15),
                    qk_u(0, hs0, 0, 1)]
            emit_attention_qs(0, 0, fill)
            fill = [qk_u(0, hs0, 0, 2), qk_u(0, hs0, 0, 3)]
            emit_attention_qs(0, 1, fill)
            fill = []
            for t_idx in range(2):
                for j in range(4):
                    fill.append(qk_u(1, hs1, t_idx, j))
            for i in range(16):
                fill.append(v_u(1, hs1, i))
            emit_attention_qs(0, 2, fill)
            emit_attention_qs(0, 3, fill)
            while fill:
                fill.pop(0)()
            emit_recip_ship(0)
            emit_collective(0)

            # load wo / res during attention(b1); reuse freed slots
            for cc in range(8):
                t = hpool.tile([128, 1024], F32R, tag=f"hs{cc}",
                               name=f"wo{cc}")
                nc.sync.dma_start(out=t[:], in_=wo[128 * cc:128 * (cc + 1), :])
                wo_sb.append(t)
            res_sb = []
            for st_i in range(4):
                t = wpool.tile([128, 1024], F32, tag=f"res{st_i}",
                               name=f"res{st_i}")
                nc.sync.dma_start(out=t[:],
                                  in_=res[128 * st_i:128 * (st_i + 1), :])
                res_sb.append(t)

            for qs in range(4):
                emit_attention_qs(1, qs, [])
            emit_recip_ship(1)
            emit_output(0, out1, res_sb, after=last_drain[0])
            emit_collective(1)
            emit_output(1, out2, res_sb)
    nc.finalize()
    return nc


def _prep_inputs(hidden_states, Wq, bq, Wk, bk, Wv, bv, Wo, bo):
    hs = np.asarray(hidden_states, np.float32)
    hsT = np.ascontiguousarray(
        hs.transpose(2, 0, 1).reshape(C, BS)).astype(np.float32)
    Wo_f = np.ascontiguousarray(np.asarray(Wo, np.float32))
    in_maps = []
    for c in range(N_CORES):
        h0 = 2 * c
        cols = slice(64 * h0, 64 * h0 + 128)
        wv_c = np.zeros((C, 256), np.float32)
        bvb_c = np.zeros((1, 256), np.float32)
        for a in range(2):
            hd = slice(64 * (h0 + a), 64 * (h0 + a + 1))
            wv_c[:, 65 * a:65 * a + 64] = np.asarray(Wv, np.float32)[:, hd]
            bvb_c[0, 65 * a:65 * a + 64] = np.asarray(bv, np.float32)[hd]
            bvb_c[0, 65 * a + 64] = 1.0
        bqk_c = np.stack([np.asarray(bq, np.float32)[cols],
                          np.asarray(bk, np.float32)[cols]], axis=1)
        b_c, s0 = c // 4, 512 * (c % 4)
        res_c = (hs[b_c, s0:s0 + 512, :] + np.asarray(bo, np.float32)
                 ).astype(np.float32)
        in_maps.append({
            "hsT": hsT,
            "wq": np.ascontiguousarray(np.asarray(Wq, np.float32)[:, cols]),
            "wk": np.ascontiguousarray(np.asarray(Wk, np.float32)[:, cols]),
            "wv": wv_c,
            "wo": Wo_f,
            "bqk": np.ascontiguousarray(bqk_c),
            "bvb": bvb_c,
            "res": np.ascontiguousarray(res_c),
        })
    return in_maps


def _run(inputs, trace=False, trace_kwargs=None):
    if "nc" not in _CACHE:
        _CACHE["nc"] = _build()
    nc = _CACHE["nc"]
    in_maps = _prep_inputs(**inputs)
    r = run_bass_kernel_spmd(nc, in_maps, core_ids=list(range(N_CORES)),
                             trace=trace, **(trace_kwargs or {}))
    full = np.empty((B, S, C), np.float32)
    for c in range(N_CORES):
        key = "out1" if c < 4 else "out2"
        full[c // 4, 512 * (c % 4):512 * (c % 4 + 1), :] = r.results[c][key]
    return full, r


def kernel(**inputs):
    full, _ = _run(inputs, trace=False)
    return full
